# revision 11
# baseline (speedup 1.0000x reference)
"""Bass/Trainium2 kernel for nn_HailNet_42975442763785 (GNN message passing).

Math insight: the COO adjacency built by the model only references node
indices in [0, 4111), and the coalesced matrix A is banded with offsets in
[-80, 80].  Therefore h1 = xf @ A.T is zero outside its first 4111 columns
and the [48,65536] @ [65536,256] embedding matmul reduces exactly to
[48,4111] @ [4111,256].  Stage A (A @ xfT) becomes a block-tridiagonal
matmul over 128-wide blocks.

Sharding (8 cores):
  - The 4111 (padded to 5120 = 40*128) contraction axis is split 5 blocks
    per core.  Stage A needs only a 1-block halo of xfT -> no communication.
  - Stage B computes per-core partial t2 pre-activations [256, 48];
    an AllGather + local tree-reduce combines them (AllGather has a much
    lower latency floor than AllReduce on 8 intra-chip ranks).
  - The tail (lin1, GRU over T=12, final MLP) is computed redundantly
    on every core; core 0's output is returned.

Perf structure vs the fp32 baseline:
  - all matmul operands in bf16 (fp32 matmul costs 4 cycles/row + no fast
    weight load; bf16 is 1 cycle/row + FWL)
  - GRU gate biases and the input projection x_t are pre-accumulated into
    PSUM with identity-weight matmuls, so the per-step critical chain is
    mm -> sigmoid(r) -> mul+add -> tanh -> mul+add with no extra bias ops
  - n-gates are computed first so the DVE r*hn product can start before
    the z-gate matmuls finish
  - all small constants ride in two packed tensors (2 DMAs instead of 10)
  - loads are spread across the SP and Pool DMA queues
"""

from contextlib import ExitStack

import numpy as np

import concourse.bass as bass
import concourse.tile as tile
from concourse import bacc, mybir
from concourse.bass_utils import run_bass_kernel_spmd

F32 = mybir.dt.float32
BF16 = mybir.dt.bfloat16
AF = mybir.ActivationFunctionType
ALU = mybir.AluOpType

N_CORES = 8
BLK = 128
NB = 5                    # I-blocks per core
NBLOCKS = N_CORES * NB    # 40 padded blocks
SUP = 4111                # true support of the adjacency
N = 65536
BT, B, T = 48, 4, 12
EMB, HID, G3 = 256, 256, 768

# wbig column layout: wl1 | wih | whh
WL1_OFF, WIH_OFF, WHH_OFF, WBIG_COLS = 0, 256, 1024, 1792


# ---------------------------------------------------------------- device code

def build_program(repeat: int = 1, loads_in_body: bool = False, coll: str = "ag",
                  t_steps: int = T):
    """coll: "ag" AllGather+local reduce (default), "ar" AllReduce,
    "none" local DMA stand-in (timing only, numerically wrong on >1 core)."""
    nc = bacc.Bacc("TRN2", target_bir_lowering=False, debug=False,
                   num_devices=N_CORES)

    at_d = nc.dram_tensor("at", [BLK, 3 * NB, BLK], BF16, kind="ExternalInput")
    xh_d = nc.dram_tensor("xh", [BLK, NB + 2, BT], BF16, kind="ExternalInput")
    wes_d = nc.dram_tensor("wes", [BLK, NB, EMB], BF16, kind="ExternalInput")
    wbig_d = nc.dram_tensor("wbig", [BLK, 2, WBIG_COLS], BF16,
                            kind="ExternalInput")
    cb_d = nc.dram_tensor("cb", [BLK, 11, 16], BF16, kind="ExternalInput")
    br_d = nc.dram_tensor("br", [1, 1328], BF16, kind="ExternalInput")
    tf_d = nc.dram_tensor("tf", [16, 20], F32, kind="ExternalInput")
    out_d = nc.dram_tensor("out", [1, B], F32, kind="ExternalOutput")

    with tile.TileContext(nc) as tc, ExitStack() as ctx:
        const = ctx.enter_context(tc.tile_pool(name="const", bufs=1))
        work = ctx.enter_context(tc.tile_pool(name="work", bufs=2))
        gru = ctx.enter_context(tc.tile_pool(name="gru", bufs=2))
        psA = ctx.enter_context(tc.tile_pool(name="psA", bufs=2, space="PSUM"))
        psB = ctx.enter_context(tc.tile_pool(name="psB", bufs=1, space="PSUM"))
        psX = ctx.enter_context(tc.tile_pool(name="psX", bufs=1, space="PSUM"))
        psG = ctx.enter_context(tc.tile_pool(name="psG", bufs=2, space="PSUM"))
        psO = ctx.enter_context(tc.tile_pool(name="psO", bufs=1, space="PSUM"))
        dram = ctx.enter_context(tc.tile_pool(name="dram", bufs=2, space="DRAM"))

        def emit_loads(pool):
            # SP queue: the stage-A gater (at, xh) then the big weight pack.
            at_sb = pool.tile([BLK, 3 * NB, BLK], BF16, tag="at_sb")
            nc.sync.dma_start(out=at_sb[:], in_=at_d[:])
            xh_sb = pool.tile([BLK, NB + 2, BT], BF16, tag="xh_sb")
            nc.sync.dma_start(out=xh_sb[:], in_=xh_d[:])
            cb_sb = pool.tile([BLK, 11, 16], BF16, tag="cb_sb")
            nc.sync.dma_start(out=cb_sb[:], in_=cb_d[:])
            wbig_sb = pool.tile([BLK, 2, WBIG_COLS], BF16, tag="wbig_sb")
            nc.sync.dma_start(out=wbig_sb[:], in_=wbig_d[:])
            cf_sb = pool.tile([BLK, 10], F32, tag="cf_sb")
            nc.sync.dma_start(out=cf_sb[:], in_=cf_d[:])
            tf_sb = pool.tile([16, 20], F32, tag="tf_sb")
            nc.sync.dma_start(out=tf_sb[:], in_=tf_d[:])
            # Pool queue: stage-B weights (parallel with at/xh on SP).
            wes_sb = pool.tile([BLK, NB, EMB], BF16, tag="wes_sb")
            nc.gpsimd.dma_start(out=wes_sb[:], in_=wes_d[:])
            return at_sb, xh_sb, wes_sb, wbig_sb, cb_sb, cf_sb, tf_sb

        if not loads_in_body:
            (at_sb, xh_sb, wes_sb, wbig_sb, cb_sb, cf_sb, tf_sb) = \
                emit_loads(const)

        # warm the ACT sigmoid/tanh table set while DMAs run
        dummy = const.tile([BLK, 1], F32)
        nc.vector.memset(dummy[:], 0.0)
        dummy2 = const.tile([BLK, 1], F32)
        nc.scalar.activation(dummy2[:], dummy[:], AF.Sigmoid)

        for _ in range(repeat):
            if loads_in_body:
                (at_sb, xh_sb, wes_sb, wbig_sb, cb_sb, cf_sb, tf_sb) = \
                    emit_loads(work)
            ident = cb_sb[:, 0:8, :]          # [128, 128] identity (bf16)
            bhn_bc = cb_sb[:, 8, 0:8]         # [128, (2c x 4b)] bhh_n bcast
            h0_c = [cb_sb[:, 8, 8:12], cb_sb[:, 8, 12:16]]   # per-kc [128, 4]
            wf0_c = [cb_sb[:, 9, :], cb_sb[:, 10, :]]        # per-kc [128, 16]

            # ---- stage A: h1T blocks [128, 48] = A @ xfT (block tridiagonal)
            h1_sb = work.tile([BLK, NB, BT], BF16)
            for i in range(NB):
                ps = psA.tile([BLK, BT], F32, tag="psa")
                for jo in range(3):
                    nc.tensor.matmul(
                        ps[:], at_sb[:, 3 * i + jo, :], xh_sb[:, i + jo, :],
                        start=(jo == 0), stop=(jo == 2))
                if i % 2 == 0:
                    nc.vector.tensor_copy(h1_sb[:, i, :], ps[:])
                else:
                    nc.scalar.activation(h1_sb[:, i, :], ps[:], AF.Copy)

            # ---- stage B: partial t2preT [256, 48] = W_es @ h1 (one PSUM tile)
            psb = psB.tile([BLK, 2, BT], F32, tag="psb")
            for e in range(2):
                for i in range(NB):
                    nc.tensor.matmul(
                        psb[:, e, :], wes_sb[:, i, e * BLK:(e + 1) * BLK],
                        h1_sb[:, i, :], start=(i == 0), stop=(i == NB - 1))
            t2p_sb = work.tile([BLK, 2, BT], BF16)
            nc.vector.tensor_copy(t2p_sb[:], psb[:])

            # ---- combine partials across the 8 cores
            if coll == "ar":
                cc_in = dram.tile([BLK, 2, BT], F32, tag="cc_in")
                cc_out = dram.tile([BLK, 2, BT], F32, tag="cc_out")
                t2pf_sb = work.tile([BLK, 2, BT], F32, tag="t2pf")
                nc.vector.tensor_copy(t2pf_sb[:], psb[:])
                nc.gpsimd.dma_start(out=cc_in[:], in_=t2pf_sb[:])
                nc.gpsimd.collective_compute(
                    "AllReduce", ALU.add,
                    replica_groups=[list(range(N_CORES))],
                    ins=[cc_in.opt()], outs=[cc_out.opt()])
                t2r_sb = work.tile([BLK, 2, BT], F32, tag="t2r")
                nc.gpsimd.dma_start(out=t2r_sb[:], in_=cc_out[:])
                t2pre = t2r_sb
            else:
                cc_in = dram.tile([BLK, 2, BT], BF16, tag="cc_in")
                cc_out = dram.tile([N_CORES, BLK, 2, BT], BF16, tag="cc_out")
                nc.gpsimd.dma_start(out=cc_in[:], in_=t2p_sb[:])
                if coll == "ag":
                    nc.gpsimd.collective_compute(
                        "AllGather", ALU.bypass,
                        replica_groups=[list(range(N_CORES))],
                        ins=[cc_in.opt()], outs=[cc_out.opt()])
                else:   # timing stand-in: local copy into slot 0
                    nc.gpsimd.dma_start(out=cc_out[0], in_=cc_in[:])
                gat_sb = work.tile([BLK, N_CORES, 2 * BT], BF16, tag="gat")
                if coll == "ag":
                    nc.gpsimd.dma_start(
                        out=gat_sb[:], in_=cc_out[:].transpose([1, 0, 2, 3]))
                else:
                    for r in range(N_CORES):
                        nc.gpsimd.dma_start(out=gat_sb[:, r, :], in_=cc_out[0])
                # tree reduce the 8 partials (f32 accumulation)
                r1 = work.tile([BLK, 4, 2 * BT], F32, tag="red1")
                nc.vector.tensor_add(r1[:], gat_sb[:, 0:4, :], gat_sb[:, 4:8, :])
                r2 = work.tile([BLK, 2, 2 * BT], F32, tag="red2")
                nc.vector.tensor_add(r2[:], r1[:, 0:2, :], r1[:, 2:4, :])
                t2r_sb = work.tile([BLK, 2, BT], F32, tag="t2r")
                nc.vector.tensor_add(t2r_sb[:], r2[:, 0, :], r2[:, 1, :])
                t2pre = t2r_sb

            # sigmoid(t2pre + b_emb)
            t2_sb = work.tile([BLK, 2, BT], BF16)
            for e in range(2):
                nc.scalar.activation(t2_sb[:, e, :], t2pre[:, e, :],
                                     AF.Sigmoid, bias=cf_sb[:, e:e + 1])

            # ---- stage C: t4T = sigmoid(W_l1 @ t2T + b_l1)
            t4_sb = work.tile([BLK, 2, BT], BF16)
            for mc in range(2):
                ps = psA.tile([BLK, BT], F32, tag="psa")
                for kc in range(2):
                    nc.tensor.matmul(
                        ps[:], wbig_sb[:, kc, WL1_OFF + mc * BLK:
                                       WL1_OFF + (mc + 1) * BLK],
                        t2_sb[:, kc, :], start=(kc == 0), stop=(kc == 1))
                nc.scalar.activation(t4_sb[:, mc, :], ps[:], AF.Sigmoid,
                                     bias=cf_sb[:, 2 + mc:3 + mc])

            # ---- stage D: xpT [128, 6, 4, 12] = W_ih @ t4T (+ gate biases)
            ps_xp = psX.tile([BLK, 6, B, T], F32)
            for c in range(6):
                for kc in range(2):
                    nc.tensor.matmul(
                        ps_xp[:, c, :, :],
                        wbig_sb[:, kc, WIH_OFF + c * BLK:WIH_OFF + (c + 1) * BLK],
                        t4_sb[:, kc, :], start=(kc == 0), stop=(kc == 1))
            # rz chunks (with combined bias) in bf16 for the PSUM seeding mms;
            # n chunks (bias bih only) kept f32 for the nin add
            xp_sb = work.tile([BLK, 4, B, T], BF16)
            xn_sb = work.tile([BLK, 2, B, T], F32)
            for c in range(4):
                if c < 2:
                    nc.vector.tensor_scalar_add(
                        xp_sb[:, c, :, :], ps_xp[:, c, :, :],
                        cf_sb[:, 4 + c:5 + c])
                else:
                    nc.scalar.activation(
                        xp_sb[:, c, :, :], ps_xp[:, c, :, :], AF.Identity,
                        bias=cf_sb[:, 4 + c:5 + c])
            nc.vector.tensor_scalar_add(
                xn_sb[:, 0, :, :], ps_xp[:, 4, :, :], cf_sb[:, 8:9])
            nc.scalar.activation(
                xn_sb[:, 1, :, :], ps_xp[:, 5, :, :], AF.Identity,
                bias=cf_sb[:, 9:10])

            # ---- GRU over T steps
            h_prev_c = h0_c                       # per-kc [128, 4] APs
            h_prev_full = cb_sb[:, 8, 8:16]       # [128, 8] AP (same data)
            for t in range(t_steps):
                ps_g = psG.tile([BLK, 6, B], F32, tag="ps_g")
                # PSUM seeds (no h dependency): xp_rz into chunks 0-3,
                # bhh_n broadcast into chunks 4-5 -- identity-weight matmuls
                for c in range(4):
                    nc.tensor.matmul(ps_g[:, c, :], ident, xp_sb[:, c, :, t],
                                     start=True, stop=False)
                for cc in range(2):
                    nc.tensor.matmul(ps_g[:, 4 + cc, :], ident,
                                     cb_sb[:, 8, 4 * cc:4 * cc + 4],
                                     start=True, stop=False)
                # W_hh @ h: n-gates first so the DVE product can start early
                for c in (4, 5, 0, 1, 2, 3):
                    for kc in range(2):
                        nc.tensor.matmul(
                            ps_g[:, c, :],
                            wbig_sb[:, kc, WHH_OFF + c * BLK:
                                    WHH_OFF + (c + 1) * BLK],
                            h_prev_c[kc], start=False, stop=(kc == 1))
                # Emission order matters: a consumer's semaphore wait covers
                # every producer-engine op emitted before it, so each op is
                # emitted as late as its true dependency allows.
                # r gate first -- npre only needs r
                r_sb = gru.tile([BLK, 2, B], F32, tag="r")
                nc.scalar.activation(r_sb[:], ps_g[:, 0:2, :], AF.Sigmoid)
                # n pre-activation: nin = xn + r * (hn + bhh_n)
                npre = gru.tile([BLK, 2, B], F32, tag="npre")
                nc.vector.tensor_mul(npre[:], ps_g[:, 4:6, :], r_sb[:])
                nin = gru.tile([BLK, 2, B], F32, tag="nin")
                nc.vector.tensor_add(nin[:], npre[:], xn_sb[:, :, :, t])
                # z gate on ACT between r and tanh (its slack window)
                z_sb = gru.tile([BLK, 2, B], F32, tag="z")
                nc.scalar.activation(z_sb[:], ps_g[:, 2:4, :], AF.Sigmoid)
                nw = gru.tile([BLK, 2, B], F32, tag="nw")
                nc.scalar.activation(nw[:], nin[:], AF.Tanh)
                # u = z*h and v = 1-z run on DVE during the tanh
                u = gru.tile([BLK, 2, B], F32, tag="u")
                nc.vector.tensor_mul(u[:], z_sb[:], h_prev_full)
                v = gru.tile([BLK, 2, B], F32, tag="v")
                nc.vector.tensor_scalar(v[:], z_sb[:], -1.0, 1.0,
                                        op0=ALU.mult, op1=ALU.add)
                w = gru.tile([BLK, 2, B], F32, tag="w")
                nc.vector.tensor_mul(w[:], nw[:], v[:])
                h_new = gru.tile([BLK, 2, B], BF16, tag="h")
                nc.vector.tensor_add(h_new[:], w[:], u[:])
                h_prev_c = [h_new[:, 0, :], h_new[:, 1, :]]
                h_prev_full = h_new[:]

            # ---- tail MLP: [4,256] -> 16 -> 16 -> 1, sigmoid each
            ps_o1 = psO.tile([16, B], F32, tag="o")
            for kc in range(2):
                nc.tensor.matmul(ps_o1[:], wf0_c[kc], h_prev_c[kc],
                                 start=(kc == 0), stop=(kc == 1))
            o1 = work.tile([16, B], F32, tag="o1s")
            nc.scalar.activation(o1[:], ps_o1[:], AF.Sigmoid,
                                 bias=tf_sb[:, 17:18])
            ps_o2 = psO.tile([16, B], F32, tag="o")
            nc.tensor.matmul(ps_o2[:], tf_sb[:, 0:16], o1[:],
                             start=True, stop=True)
            o2 = work.tile([16, B], F32, tag="o2s")
            nc.scalar.activation(o2[:], ps_o2[:], AF.Sigmoid,
                                 bias=tf_sb[:, 18:19])
            ps_o3 = psO.tile([1, B], F32, tag="o")
            nc.tensor.matmul(ps_o3[:], tf_sb[:, 16:17], o2[:],
                             start=True, stop=True)
            o3 = work.tile([1, B], F32, tag="o3s")
            nc.scalar.activation(o3[:], ps_o3[:], AF.Sigmoid,
                                 bias=tf_sb[0:1, 19:20])
            nc.sync.dma_start(out=out_d[:], in_=o3[:])

    nc.compile()
    return nc


# ---------------------------------------------------------------- host side

def prepare_in_maps(x, h0, rows, cols, W_emb, b_emb, W_l1, b_l1,
                    W_ih, W_hh, b_ih, b_hh, W_f0, b_f0, W_f1, b_f1,
                    W_f2, b_f2):
    import ml_dtypes
    f32 = np.float32
    bf = ml_dtypes.bfloat16
    x = np.ascontiguousarray(x, f32)
    assert int(rows.max()) < SUP and int(cols.max()) < SUP

    # dense banded adjacency on its true support (duplicates sum = coalesce)
    A = np.zeros((SUP, SUP), f32)
    np.add.at(A, (np.asarray(rows), np.asarray(cols)), 1.0)

    S_pad = NBLOCKS * BLK
    ATp = np.zeros((S_pad, S_pad), f32)
    ATp[:SUP, :SUP] = A.T

    xf = x.reshape(BT, N)
    # xsT padded with one leading zero block (halo for core 0) + tail blocks
    XTp = np.zeros(((NBLOCKS + 2) * BLK, BT), f32)
    XTp[BLK:BLK + SUP] = xf[:, :SUP].T

    WesT = np.zeros((S_pad, EMB), f32)
    WesT[:SUP] = np.asarray(W_emb, f32)[:, :SUP].T

    def pm(vec, k):  # partition-major [128, k] view of a length 128*k vector
        return np.ascontiguousarray(np.asarray(vec, f32).reshape(k, BLK).T)

    def pm3(w, k):  # [M, K] weight -> partition-major [128, K//128, M]
        return np.ascontiguousarray(
            np.asarray(w, f32).T.reshape(-1, BLK, k).transpose(1, 0, 2))

    bih = np.asarray(b_ih, f32)
    bhh = np.asarray(b_hh, f32)
    bxp = np.concatenate([bih[:512] + bhh[:512], bih[512:]])  # rz: both, n: ih

    # wbig pack [128, 2, 1792]: wl1 | wih | whh
    wbig = np.zeros((BLK, 2, WBIG_COLS), f32)
    wbig[:, :, WL1_OFF:WL1_OFF + EMB] = pm3(W_l1, EMB)
    wbig[:, :, WIH_OFF:WIH_OFF + G3] = pm3(W_ih, G3)
    wbig[:, :, WHH_OFF:WHH_OFF + G3] = pm3(W_hh, G3)

    # cb pack [128, 11, 16] bf16: identity | bhn_bc + h0 | wf0 (2 chunks)
    cb = np.zeros((BLK, 11, 16), f32)
    cb[:, 0:8, :] = np.eye(BLK, dtype=f32).reshape(BLK, 8, 16)
    bhn = bhh[512:].reshape(2, BLK)                    # [c, p]
    cb[:, 8, 0:8] = np.repeat(bhn.T, B, axis=1).reshape(BLK, 8)
    h0c = np.asarray(h0, f32)[0].T.reshape(2, BLK, B)  # [kc, p, b]
    cb[:, 8, 8:16] = h0c.transpose(1, 0, 2).reshape(BLK, 8)
    wf0t = pm3(W_f0, 16)                               # [128, 2, 16]
    cb[:, 9, :] = wf0t[:, 0, :]
    cb[:, 10, :] = wf0t[:, 1, :]

    # cf pack [128, 10] f32: bemb | bl1 | bxp
    cf = np.zeros((BLK, 10), f32)
    cf[:, 0:2] = pm(b_emb, 2)
    cf[:, 2:4] = pm(b_l1, 2)
    cf[:, 4:10] = pm(bxp, 6)

    # tf pack [16, 20] f32: wf1 | wf2t | bf0 | bf1 | bf2
    tf = np.zeros((16, 20), f32)
    tf[:, 0:16] = np.asarray(W_f1, f32).T
    tf[:, 16] = np.asarray(W_f2, f32).reshape(16)
    tf[:, 17] = np.asarray(b_f0, f32)
    tf[:, 18] = np.asarray(b_f1, f32)
    tf[0, 19] = np.asarray(b_f2, f32).reshape(1)[0]

    common = dict(wbig=wbig.astype(bf), cb=cb.astype(bf), cf=cf, tf=tf)

    in_maps = []
    for c in range(N_CORES):
        at = np.zeros((3 * NB, BLK, BLK), f32)
        for i in range(NB):
            I = NB * c + i
            for jo in range(3):
                J = I - 1 + jo
                if 0 <= J < NBLOCKS:
                    at[3 * i + jo] = ATp[J * BLK:(J + 1) * BLK,
                                         I * BLK:(I + 1) * BLK]
        xh = np.ascontiguousarray(
            XTp[NB * c * BLK:(NB * c + NB + 2) * BLK]
            .reshape(NB + 2, BLK, BT).transpose(1, 0, 2))
        wes = np.ascontiguousarray(
            WesT[NB * c * BLK:(NB * (c + 1)) * BLK]
            .reshape(NB, BLK, EMB).transpose(1, 0, 2))
        in_maps.append(dict(
            at=np.ascontiguousarray(at.transpose(1, 0, 2)).astype(bf),
            xh=xh.astype(bf), wes=wes.astype(bf), **common))
    return in_maps


# production configuration for kernel(); test.py reads this too
KERNEL_CONFIG = dict(coll="ag")

_CACHE = {}


def kernel(**inputs) -> np.ndarray:
    if "nc" not in _CACHE:
        _CACHE["nc"] = build_program(**KERNEL_CONFIG)
    nc = _CACHE["nc"]
    in_maps = prepare_in_maps(**inputs)
    res = run_bass_kernel_spmd(nc, in_maps, list(range(N_CORES)))
    out = res.results[0]["out"]          # [1, 4]
    return np.ascontiguousarray(out.T.astype(np.float32))  # [4, 1]


if __name__ == "__main__":
    import importlib.util
    spec = importlib.util.spec_from_file_location("reference", "reference.py")
    ref = importlib.util.module_from_spec(spec)
    spec.loader.exec_module(ref)
    inputs = {k: np.asarray(v) for k, v in ref.setup_inputs().items()}
    expected = np.asarray(ref.reference(**inputs))
    got = kernel(**inputs)
    err = np.abs(got - expected).max() / np.abs(expected).max()
    print("expected:", expected.ravel())
    print("got:     ", got.ravel())
    print("Relative error:", err)


# revision 16
# speedup vs baseline: 4.4248x; 4.4248x over previous
"""Bass/Trainium2 kernel for nn_HailNet_42975442763785 (GNN message passing).

Math insight: the COO adjacency built by the model only references node
indices in [0, 4111), and the coalesced matrix A is banded with offsets in
[-80, 80].  Therefore h1 = xf @ A.T is zero outside its first 4111 columns
and the [48,65536] @ [65536,256] embedding matmul reduces exactly to
[48,4111] @ [4111,256].  Stage A (A @ xfT) becomes a block-tridiagonal
matmul over 128-wide blocks.

Sharding (8 cores):
  - The 4111 (padded to 5120 = 40*128) contraction axis is split 5 blocks
    per core.  Stage A needs only a 1-block halo of xfT -> no communication.
  - Stage B computes per-core partial t2 pre-activations [256, 48];
    an AllGather + local tree-reduce combines them (AllGather has a much
    lower latency floor than AllReduce on 8 intra-chip ranks).
  - The tail (lin1, GRU over T=12, final MLP) is computed redundantly
    on every core; core 0's output is returned.

Perf structure vs the fp32 baseline:
  - all matmul operands in bf16 (fp32 matmul costs 4 cycles/row + no fast
    weight load; bf16 is 1 cycle/row + FWL)
  - GRU gate biases and the input projection x_t are pre-accumulated into
    PSUM with identity-weight matmuls, and all linear-stage biases are
    seeded into PSUM with ones-row rank-1 matmuls, so the per-step
    critical chain is mm -> sigmoid(rz) -> mul+add -> tanh -> mul+add
    with no separate bias ops anywhere
  - emission order is tuned for the per-engine program-order semaphore
    counters (each consumer waits only for what it truly needs)
  - all small constants ride in packed tensors (few big DMAs), spread
    across the SP and Pool DMA queues
"""

from contextlib import ExitStack

import numpy as np

import concourse.bass as bass
import concourse.tile as tile
from concourse import bacc, mybir
from concourse.bass_utils import run_bass_kernel_spmd

F32 = mybir.dt.float32
BF16 = mybir.dt.bfloat16
AF = mybir.ActivationFunctionType
ALU = mybir.AluOpType

N_CORES = 8
BLK = 128
NB = 5                    # I-blocks per core
NBLOCKS = N_CORES * NB    # 40 padded blocks
SUP = 4111                # true support of the adjacency
N = 65536
BT, B, T = 48, 4, 12
EMB, HID, G3 = 256, 256, 768

# wbig column layout: wl1 | wih | whh
WL1_OFF, WIH_OFF, WHH_OFF, WBIG_COLS = 0, 256, 1024, 1792


# ---------------------------------------------------------------- device code

def build_program(repeat: int = 1, loads_in_body: bool = False, coll: str = "ag",
                  t_steps: int = T):
    """coll: "ag" AllGather+local reduce (default), "ar" AllReduce,
    "none" local DMA stand-in (timing only, numerically wrong on >1 core)."""
    nc = bacc.Bacc("TRN2", target_bir_lowering=False, debug=False,
                   num_devices=N_CORES)

    at_d = nc.dram_tensor("at", [BLK, 3 * NB, BLK], BF16, kind="ExternalInput")
    xh_d = nc.dram_tensor("xh", [BLK, NB + 2, BT], BF16, kind="ExternalInput")
    wes_d = nc.dram_tensor("wes", [BLK, NB, EMB], BF16, kind="ExternalInput")
    wbig_d = nc.dram_tensor("wbig", [BLK, 2, WBIG_COLS], BF16,
                            kind="ExternalInput")
    cb_d = nc.dram_tensor("cb", [BLK, 11, 16], BF16, kind="ExternalInput")
    br_d = nc.dram_tensor("br", [1, 1328], BF16, kind="ExternalInput")
    tf_d = nc.dram_tensor("tf", [16, 20], F32, kind="ExternalInput")
    out_d = nc.dram_tensor("out", [1, B], F32, kind="ExternalOutput")

    with tile.TileContext(nc) as tc, ExitStack() as ctx:
        const = ctx.enter_context(tc.tile_pool(name="const", bufs=1))
        work = ctx.enter_context(tc.tile_pool(name="work", bufs=2))
        gru = ctx.enter_context(tc.tile_pool(name="gru", bufs=2))
        psA = ctx.enter_context(tc.tile_pool(name="psA", bufs=2, space="PSUM"))
        psB = ctx.enter_context(tc.tile_pool(name="psB", bufs=1, space="PSUM"))
        psX = ctx.enter_context(tc.tile_pool(name="psX", bufs=1, space="PSUM"))
        psG = ctx.enter_context(tc.tile_pool(name="psG", bufs=2, space="PSUM"))
        psO = ctx.enter_context(tc.tile_pool(name="psO", bufs=1, space="PSUM"))
        dram = ctx.enter_context(tc.tile_pool(name="dram", bufs=2, space="DRAM"))

        def emit_loads(pool):
            # SP queue: the stage-A gater (at, xh) then the big weight pack.
            at_sb = pool.tile([BLK, 3 * NB, BLK], BF16, tag="at_sb")
            nc.sync.dma_start(out=at_sb[:], in_=at_d[:])
            xh_sb = pool.tile([BLK, NB + 2, BT], BF16, tag="xh_sb")
            nc.sync.dma_start(out=xh_sb[:], in_=xh_d[:])
            cb_sb = pool.tile([BLK, 11, 16], BF16, tag="cb_sb")
            nc.sync.dma_start(out=cb_sb[:], in_=cb_d[:])
            br_sb = pool.tile([1, 1328], BF16, tag="br_sb")
            nc.sync.dma_start(out=br_sb[:], in_=br_d[:])
            wbig_sb = pool.tile([BLK, 2, WBIG_COLS], BF16, tag="wbig_sb")
            nc.sync.dma_start(out=wbig_sb[:], in_=wbig_d[:])
            tf_sb = pool.tile([16, 20], F32, tag="tf_sb")
            nc.sync.dma_start(out=tf_sb[:], in_=tf_d[:])
            # Pool queue: stage-B weights (parallel with at/xh on SP).
            wes_sb = pool.tile([BLK, NB, EMB], BF16, tag="wes_sb")
            nc.gpsimd.dma_start(out=wes_sb[:], in_=wes_d[:])
            return at_sb, xh_sb, wes_sb, wbig_sb, cb_sb, br_sb, tf_sb

        if not loads_in_body:
            (at_sb, xh_sb, wes_sb, wbig_sb, cb_sb, br_sb, tf_sb) = \
                emit_loads(const)

        # warm the ACT sigmoid/tanh table set while DMAs run
        dummy = const.tile([BLK, 1], F32)
        nc.vector.memset(dummy[:], 0.0)
        dummy2 = const.tile([BLK, 1], F32)
        nc.scalar.activation(dummy2[:], dummy[:], AF.Sigmoid)

        for _ in range(repeat):
            if loads_in_body:
                (at_sb, xh_sb, wes_sb, wbig_sb, cb_sb, br_sb, tf_sb) = \
                    emit_loads(work)
            ident = cb_sb[:, 0:8, :]          # [128, 128] identity (bf16)
            bhn_bc = cb_sb[:, 8, 0:8]         # [128, (2c x 4b)] bhh_n bcast
            h0_c = [cb_sb[:, 8, 8:12], cb_sb[:, 8, 12:16]]   # per-kc [128, 4]
            wf0_c = [cb_sb[:, 9, :], cb_sb[:, 10, :]]        # per-kc [128, 16]
            ones48 = br_sb[0:1, 1280:1328]                   # [1, 48] of 1.0
            def brow(k):                                     # [1, 128] bias row
                return br_sb[0:1, k * BLK:(k + 1) * BLK]

            # ---- stage A: h1T blocks [128, 48] = A @ xfT (block tridiagonal)
            h1_sb = work.tile([BLK, NB, BT], BF16)
            for i in range(NB):
                ps = psA.tile([BLK, BT], F32, tag="psa")
                for jo in range(3):
                    nc.tensor.matmul(
                        ps[:], at_sb[:, 3 * i + jo, :], xh_sb[:, i + jo, :],
                        start=(jo == 0), stop=(jo == 2))
                if i % 2 == 0:
                    nc.vector.tensor_copy(h1_sb[:, i, :], ps[:])
                else:
                    nc.scalar.activation(h1_sb[:, i, :], ps[:], AF.Copy)

            # ---- stage B: partial t2preT [256, 48] = W_es @ h1 (one PSUM tile)
            psb = psB.tile([BLK, 2, BT], F32, tag="psb")
            for e in range(2):
                for i in range(NB):
                    nc.tensor.matmul(
                        psb[:, e, :], wes_sb[:, i, e * BLK:(e + 1) * BLK],
                        h1_sb[:, i, :], start=(i == 0), stop=False)
                nc.tensor.matmul(psb[:, e, :], brow(e), ones48,
                                 start=False, stop=True)
            t2p_sb = work.tile([BLK, 2, BT], BF16)
            nc.vector.tensor_copy(t2p_sb[:], psb[:])

            # ---- combine partials across the 8 cores
            if coll == "ar":
                cc_in = dram.tile([BLK, 2, BT], F32, tag="cc_in")
                cc_out = dram.tile([BLK, 2, BT], F32, tag="cc_out")
                t2pf_sb = work.tile([BLK, 2, BT], F32, tag="t2pf")
                nc.vector.tensor_copy(t2pf_sb[:], psb[:])
                nc.gpsimd.dma_start(out=cc_in[:], in_=t2pf_sb[:])
                nc.gpsimd.collective_compute(
                    "AllReduce", ALU.add,
                    replica_groups=[list(range(N_CORES))],
                    ins=[cc_in.opt()], outs=[cc_out.opt()])
                t2r_sb = work.tile([BLK, 2, BT], F32, tag="t2r")
                nc.gpsimd.dma_start(out=t2r_sb[:], in_=cc_out[:])
                t2pre = t2r_sb
            else:
                cc_in = dram.tile([BLK, 2, BT], BF16, tag="cc_in")
                cc_out = dram.tile([N_CORES, BLK, 2, BT], BF16, tag="cc_out")
                nc.gpsimd.dma_start(out=cc_in[:], in_=t2p_sb[:])
                if coll == "ag":
                    nc.gpsimd.collective_compute(
                        "AllGather", ALU.bypass,
                        replica_groups=[list(range(N_CORES))],
                        ins=[cc_in.opt()], outs=[cc_out.opt()])
                else:   # timing stand-in: local copy into slot 0
                    nc.gpsimd.dma_start(out=cc_out[0], in_=cc_in[:])
                gat_sb = work.tile([BLK, N_CORES, 2 * BT], BF16, tag="gat")
                if coll == "ag":
                    nc.gpsimd.dma_start(
                        out=gat_sb[:], in_=cc_out[:].transpose([1, 0, 2, 3]))
                else:
                    for r in range(N_CORES):
                        nc.gpsimd.dma_start(out=gat_sb[:, r, :], in_=cc_out[0])
                # tree reduce the 8 partials (f32 accumulation)
                r1 = work.tile([BLK, 4, 2 * BT], BF16, tag="red1")
                nc.vector.tensor_add(r1[:], gat_sb[:, 0:4, :], gat_sb[:, 4:8, :])
                r2 = work.tile([BLK, 2, 2 * BT], BF16, tag="red2")
                nc.vector.tensor_add(r2[:], r1[:, 0:2, :], r1[:, 2:4, :])
                t2r_sb = work.tile([BLK, 2, BT], F32, tag="t2r")
                nc.vector.tensor_add(t2r_sb[:], r2[:, 0, :], r2[:, 1, :])
                t2pre = t2r_sb

            # sigmoid(t2pre); b_emb/8 was seeded into each partial pre-gather
            t2_sb = work.tile([BLK, 2, BT], BF16)
            nc.scalar.activation(t2_sb[:], t2pre[:], AF.Sigmoid)

            # ---- stage C: t4T = sigmoid(W_l1 @ t2T + b_l1)
            psc = psB.tile([BLK, 2, BT], F32, tag="psc")
            for mc in range(2):
                nc.tensor.matmul(psc[:, mc, :], brow(2 + mc), ones48,
                                 start=True, stop=False)
                for kc in range(2):
                    nc.tensor.matmul(
                        psc[:, mc, :], wbig_sb[:, kc, WL1_OFF + mc * BLK:
                                               WL1_OFF + (mc + 1) * BLK],
                        t2_sb[:, kc, :], start=False, stop=(kc == 1))
            t4_sb = work.tile([BLK, 2, BT], BF16)
            nc.scalar.activation(t4_sb[:], psc[:], AF.Sigmoid)

            # ---- stage D: xpT [128, 6, 4, 12] = W_ih @ t4T (+ gate biases)
            ps_xp = psX.tile([BLK, 6, B, T], F32)
            for c in range(6):
                nc.tensor.matmul(ps_xp[:, c, :, :], brow(4 + c), ones48,
                                 start=True, stop=False)
                for kc in range(2):
                    nc.tensor.matmul(
                        ps_xp[:, c, :, :],
                        wbig_sb[:, kc, WIH_OFF + c * BLK:WIH_OFF + (c + 1) * BLK],
                        t4_sb[:, kc, :], start=False, stop=(kc == 1))
            # rz chunks (bias included) in bf16 for the PSUM seeding mms;
            # n chunks (bias bih only) kept f32 for the nin add
            xp_sb = work.tile([BLK, 4, B, T], BF16)
            nc.vector.tensor_copy(xp_sb[:], ps_xp[:, 0:4, :, :])
            xn_sb = work.tile([BLK, 2, B, T], F32)
            nc.scalar.activation(xn_sb[:], ps_xp[:, 4:6, :, :], AF.Copy)

            # ---- GRU over T steps
            h_prev_c = h0_c                       # per-kc [128, 4] APs
            h_prev_full = cb_sb[:, 8, 8:16]       # [128, 8] AP (same data)
            for t in range(t_steps):
                ps_g = psG.tile([BLK, 6, B], F32, tag="ps_g")
                # PSUM seeds (no h dependency): xp_rz into chunks 0-3,
                # bhh_n broadcast into chunks 4-5 -- identity-weight matmuls
                for c in range(4):
                    nc.tensor.matmul(ps_g[:, c, :], ident, xp_sb[:, c, :, t],
                                     start=True, stop=False)
                for cc in range(2):
                    nc.tensor.matmul(ps_g[:, 4 + cc, :], ident,
                                     cb_sb[:, 8, 4 * cc:4 * cc + 4],
                                     start=True, stop=False)
                # W_hh @ h: rz-gate chunks first so the merged sigmoid can
                # start while the n-gate matmuls finish
                for c in (0, 1, 2, 3, 4, 5):
                    for kc in range(2):
                        nc.tensor.matmul(
                            ps_g[:, c, :],
                            wbig_sb[:, kc, WHH_OFF + c * BLK:
                                    WHH_OFF + (c + 1) * BLK],
                            h_prev_c[kc], start=False, stop=(kc == 1))
                # Emission order matters: a consumer's semaphore wait covers
                # every producer-engine op emitted before it, so each op is
                # emitted as late as its true dependency allows.
                rz_sb = gru.tile([BLK, 4, B], F32, tag="rz")
                nc.scalar.activation(rz_sb[:], ps_g[:, 0:4, :], AF.Sigmoid)
                # n pre-activation: nin = xn + r * (hn + bhh_n)
                npre = gru.tile([BLK, 2, B], F32, tag="npre")
                nc.vector.tensor_mul(npre[:], ps_g[:, 4:6, :], rz_sb[:, 0:2, :])
                nin = gru.tile([BLK, 2, B], F32, tag="nin")
                nc.vector.tensor_add(nin[:], npre[:], xn_sb[:, :, :, t])
                nw = gru.tile([BLK, 2, B], F32, tag="nw")
                nc.scalar.activation(nw[:], nin[:], AF.Tanh)
                # u = z*h and v = 1-z run on DVE during the tanh
                u = gru.tile([BLK, 2, B], F32, tag="u")
                nc.vector.tensor_mul(u[:], rz_sb[:, 2:4, :], h_prev_full)
                v = gru.tile([BLK, 2, B], F32, tag="v")
                nc.vector.tensor_scalar(v[:], rz_sb[:, 2:4, :], -1.0, 1.0,
                                        op0=ALU.mult, op1=ALU.add)
                w = gru.tile([BLK, 2, B], F32, tag="w")
                nc.vector.tensor_mul(w[:], nw[:], v[:])
                h_new = gru.tile([BLK, 2, B], BF16, tag="h")
                nc.vector.tensor_add(h_new[:], w[:], u[:])
                h_prev_c = [h_new[:, 0, :], h_new[:, 1, :]]
                h_prev_full = h_new[:]

            # ---- tail MLP: [4,256] -> 16 -> 16 -> 1, sigmoid each
            ps_o1 = psO.tile([16, B], F32, tag="o")
            for kc in range(2):
                nc.tensor.matmul(ps_o1[:], wf0_c[kc], h_prev_c[kc],
                                 start=(kc == 0), stop=(kc == 1))
            o1 = work.tile([16, B], F32, tag="o1s")
            nc.scalar.activation(o1[:], ps_o1[:], AF.Sigmoid,
                                 bias=tf_sb[:, 17:18])
            ps_o2 = psO.tile([16, B], F32, tag="o")
            nc.tensor.matmul(ps_o2[:], tf_sb[:, 0:16], o1[:],
                             start=True, stop=True)
            o2 = work.tile([16, B], F32, tag="o2s")
            nc.scalar.activation(o2[:], ps_o2[:], AF.Sigmoid,
                                 bias=tf_sb[:, 18:19])
            ps_o3 = psO.tile([1, B], F32, tag="o")
            nc.tensor.matmul(ps_o3[:], tf_sb[:, 16:17], o2[:],
                             start=True, stop=True)
            o3 = work.tile([1, B], F32, tag="o3s")
            nc.scalar.activation(o3[:], ps_o3[:], AF.Sigmoid,
                                 bias=tf_sb[0:1, 19:20])
            nc.sync.dma_start(out=out_d[:], in_=o3[:])

    nc.compile()
    return nc


# ---------------------------------------------------------------- host side

def prepare_in_maps(x, h0, rows, cols, W_emb, b_emb, W_l1, b_l1,
                    W_ih, W_hh, b_ih, b_hh, W_f0, b_f0, W_f1, b_f1,
                    W_f2, b_f2):
    import ml_dtypes
    f32 = np.float32
    bf = ml_dtypes.bfloat16
    x = np.ascontiguousarray(x, f32)
    assert int(rows.max()) < SUP and int(cols.max()) < SUP

    # dense banded adjacency on its true support (duplicates sum = coalesce)
    A = np.zeros((SUP, SUP), f32)
    np.add.at(A, (np.asarray(rows), np.asarray(cols)), 1.0)

    S_pad = NBLOCKS * BLK
    ATp = np.zeros((S_pad, S_pad), f32)
    ATp[:SUP, :SUP] = A.T

    xf = x.reshape(BT, N)
    # xsT padded with one leading zero block (halo for core 0) + tail blocks
    XTp = np.zeros(((NBLOCKS + 2) * BLK, BT), f32)
    XTp[BLK:BLK + SUP] = xf[:, :SUP].T

    WesT = np.zeros((S_pad, EMB), f32)
    WesT[:SUP] = np.asarray(W_emb, f32)[:, :SUP].T

    def pm(vec, k):  # partition-major [128, k] view of a length 128*k vector
        return np.ascontiguousarray(np.asarray(vec, f32).reshape(k, BLK).T)

    def pm3(w, k):  # [M, K] weight -> partition-major [128, K//128, M]
        return np.ascontiguousarray(
            np.asarray(w, f32).T.reshape(-1, BLK, k).transpose(1, 0, 2))

    bih = np.asarray(b_ih, f32)
    bhh = np.asarray(b_hh, f32)
    bxp = np.concatenate([bih[:512] + bhh[:512], bih[512:]])  # rz: both, n: ih

    # wbig pack [128, 2, 1792]: wl1 | wih | whh
    wbig = np.zeros((BLK, 2, WBIG_COLS), f32)
    wbig[:, :, WL1_OFF:WL1_OFF + EMB] = pm3(W_l1, EMB)
    wbig[:, :, WIH_OFF:WIH_OFF + G3] = pm3(W_ih, G3)
    wbig[:, :, WHH_OFF:WHH_OFF + G3] = pm3(W_hh, G3)

    # cb pack [128, 11, 16] bf16: identity | bhn_bc + h0 | wf0 (2 chunks)
    cb = np.zeros((BLK, 11, 16), f32)
    cb[:, 0:8, :] = np.eye(BLK, dtype=f32).reshape(BLK, 8, 16)
    bhn = bhh[512:].reshape(2, BLK)                    # [c, p]
    cb[:, 8, 0:8] = np.repeat(bhn.T, B, axis=1).reshape(BLK, 8)
    h0c = np.asarray(h0, f32)[0].T.reshape(2, BLK, B)  # [kc, p, b]
    cb[:, 8, 8:16] = h0c.transpose(1, 0, 2).reshape(BLK, 8)
    wf0t = pm3(W_f0, 16)                               # [128, 2, 16]
    cb[:, 9, :] = wf0t[:, 0, :]
    cb[:, 10, :] = wf0t[:, 1, :]

    # br pack [1, 1328] bf16: bias rows for the ones-matmul PSUM seeds.
    # bemb is divided by 8 because every core seeds it into its partial and
    # the cross-core reduce sums 8 copies.
    br = np.zeros((1, 1328), f32)
    br[0, 0:256] = np.asarray(b_emb, f32) / 8.0
    br[0, 256:512] = np.asarray(b_l1, f32)
    br[0, 512:1280] = bxp
    br[0, 1280:1328] = 1.0

    # tf pack [16, 20] f32: wf1 | wf2t | bf0 | bf1 | bf2
    tf = np.zeros((16, 20), f32)
    tf[:, 0:16] = np.asarray(W_f1, f32).T
    tf[:, 16] = np.asarray(W_f2, f32).reshape(16)
    tf[:, 17] = np.asarray(b_f0, f32)
    tf[:, 18] = np.asarray(b_f1, f32)
    tf[0, 19] = np.asarray(b_f2, f32).reshape(1)[0]

    common = dict(wbig=wbig.astype(bf), cb=cb.astype(bf), br=br.astype(bf),
                  tf=tf)

    in_maps = []
    for c in range(N_CORES):
        at = np.zeros((3 * NB, BLK, BLK), f32)
        for i in range(NB):
            I = NB * c + i
            for jo in range(3):
                J = I - 1 + jo
                if 0 <= J < NBLOCKS:
                    at[3 * i + jo] = ATp[J * BLK:(J + 1) * BLK,
                                         I * BLK:(I + 1) * BLK]
        xh = np.ascontiguousarray(
            XTp[NB * c * BLK:(NB * c + NB + 2) * BLK]
            .reshape(NB + 2, BLK, BT).transpose(1, 0, 2))
        wes = np.ascontiguousarray(
            WesT[NB * c * BLK:(NB * (c + 1)) * BLK]
            .reshape(NB, BLK, EMB).transpose(1, 0, 2))
        in_maps.append(dict(
            at=np.ascontiguousarray(at.transpose(1, 0, 2)).astype(bf),
            xh=xh.astype(bf), wes=wes.astype(bf), **common))
    return in_maps


# production configuration for kernel(); test.py reads this too
KERNEL_CONFIG = dict(coll="ag")

_CACHE = {}


def kernel(**inputs) -> np.ndarray:
    if "nc" not in _CACHE:
        _CACHE["nc"] = build_program(**KERNEL_CONFIG)
    nc = _CACHE["nc"]
    in_maps = prepare_in_maps(**inputs)
    res = run_bass_kernel_spmd(nc, in_maps, list(range(N_CORES)))
    out = res.results[0]["out"]          # [1, 4]
    return np.ascontiguousarray(out.T.astype(np.float32))  # [4, 1]


if __name__ == "__main__":
    import importlib.util
    spec = importlib.util.spec_from_file_location("reference", "reference.py")
    ref = importlib.util.module_from_spec(spec)
    spec.loader.exec_module(ref)
    inputs = {k: np.asarray(v) for k, v in ref.setup_inputs().items()}
    expected = np.asarray(ref.reference(**inputs))
    got = kernel(**inputs)
    err = np.abs(got - expected).max() / np.abs(expected).max()
    print("expected:", expected.ravel())
    print("got:     ", got.ravel())
    print("Relative error:", err)


# revision 18
# speedup vs baseline: 6.4512x; 1.4580x over previous
"""Bass/Trainium2 kernel for nn_HailNet_42975442763785 (GNN message passing).

Math insight: the COO adjacency built by the model only references node
indices in [0, 4111), and the coalesced matrix A is banded with offsets in
[-80, 80].  Therefore h1 = xf @ A.T is zero outside its first 4111 columns
and the [48,65536] @ [65536,256] embedding matmul reduces exactly to
[48,4111] @ [4111,256].  Stage A (A @ xfT) becomes a block-tridiagonal
matmul over 128-wide blocks.

Sharding (8 cores):
  - The 4111 (padded to 5120 = 40*128) contraction axis is split 5 blocks
    per core.  Stage A needs only a 1-block halo of xfT -> no communication.
  - Stage B computes per-core partial t2 pre-activations [256, 48];
    an AllGather + local tree-reduce combines them (AllGather has a much
    lower latency floor than AllReduce on 8 intra-chip ranks).
  - The tail (lin1, GRU over T=12, final MLP) is computed redundantly
    on every core; core 0's output is returned.

Perf structure vs the fp32 baseline:
  - all matmul operands in bf16 (fp32 matmul costs 4 cycles/row + no fast
    weight load; bf16 is 1 cycle/row + FWL)
  - GRU gate biases and the input projection x_t are pre-accumulated into
    PSUM with identity-weight matmuls, and all linear-stage biases are
    seeded into PSUM with ones-row rank-1 matmuls, so the per-step
    critical chain is mm -> sigmoid(rz) -> mul+add -> tanh -> mul+add
    with no separate bias ops anywhere
  - emission order is tuned for the per-engine program-order semaphore
    counters (each consumer waits only for what it truly needs)
  - all small constants ride in packed tensors (few big DMAs), spread
    across the SP and Pool DMA queues
"""

from contextlib import ExitStack

import numpy as np

import concourse.bass as bass
import concourse.tile as tile
from concourse import bacc, mybir
from concourse.bass_utils import run_bass_kernel_spmd

F32 = mybir.dt.float32
BF16 = mybir.dt.bfloat16
FP8 = mybir.dt.float8e4
AF = mybir.ActivationFunctionType
ALU = mybir.AluOpType

N_CORES = 8
BLK = 128
NB = 5                    # I-blocks per core
NBLOCKS = N_CORES * NB    # 40 padded blocks
SUP = 4111                # true support of the adjacency
N = 65536
BT, B, T = 48, 4, 12
EMB, HID, G3 = 256, 256, 768

# wbig column layout: wl1 | wih  (whh rides separately in fp8, scaled x8)
WL1_OFF, WIH_OFF, WBIG_COLS = 0, 256, 1024
WHH_SCALE = 8.0


# ---------------------------------------------------------------- device code

def build_program(repeat: int = 1, loads_in_body: bool = False, coll: str = "ag",
                  t_steps: int = T):
    """coll: "ag" AllGather+local reduce (default), "ar" AllReduce,
    "none" local DMA stand-in (timing only, numerically wrong on >1 core)."""
    nc = bacc.Bacc("TRN2", target_bir_lowering=False, debug=False,
                   num_devices=N_CORES)

    at_d = nc.dram_tensor("at", [BLK, 3 * NB, BLK], BF16, kind="ExternalInput")
    xh_d = nc.dram_tensor("xh", [BLK, NB + 2, BT], BF16, kind="ExternalInput")
    wes_d = nc.dram_tensor("wes", [BLK, NB, EMB], BF16, kind="ExternalInput")
    wbig_d = nc.dram_tensor("wbig", [BLK, 2, WBIG_COLS], BF16,
                            kind="ExternalInput")
    whh8_d = nc.dram_tensor("whh8", [BLK, 2, G3], FP8, kind="ExternalInput")
    cb_d = nc.dram_tensor("cb", [BLK, 11, 16], BF16, kind="ExternalInput")
    br_d = nc.dram_tensor("br", [1, 1328], BF16, kind="ExternalInput")
    tf_d = nc.dram_tensor("tf", [16, 20], F32, kind="ExternalInput")
    out_d = nc.dram_tensor("out", [1, B], F32, kind="ExternalOutput")

    with tile.TileContext(nc) as tc, ExitStack() as ctx:
        const = ctx.enter_context(tc.tile_pool(name="const", bufs=1))
        work = ctx.enter_context(tc.tile_pool(name="work", bufs=2))
        gru = ctx.enter_context(tc.tile_pool(name="gru", bufs=2))
        psA = ctx.enter_context(tc.tile_pool(name="psA", bufs=2, space="PSUM"))
        psB = ctx.enter_context(tc.tile_pool(name="psB", bufs=1, space="PSUM"))
        psX = ctx.enter_context(tc.tile_pool(name="psX", bufs=1, space="PSUM"))
        psG = ctx.enter_context(tc.tile_pool(name="psG", bufs=2, space="PSUM"))
        psO = ctx.enter_context(tc.tile_pool(name="psO", bufs=1, space="PSUM"))
        dram = ctx.enter_context(tc.tile_pool(name="dram", bufs=2, space="DRAM"))

        def emit_loads(pool):
            # SP queue: the stage-A gater (at, xh) then the big weight pack.
            at_sb = pool.tile([BLK, 3 * NB, BLK], BF16, tag="at_sb")
            nc.sync.dma_start(out=at_sb[:], in_=at_d[:])
            xh_sb = pool.tile([BLK, NB + 2, BT], BF16, tag="xh_sb")
            nc.sync.dma_start(out=xh_sb[:], in_=xh_d[:])
            cb_sb = pool.tile([BLK, 11, 16], BF16, tag="cb_sb")
            nc.sync.dma_start(out=cb_sb[:], in_=cb_d[:])
            br_sb = pool.tile([1, 1328], BF16, tag="br_sb")
            nc.sync.dma_start(out=br_sb[:], in_=br_d[:])
            wbig_sb = pool.tile([BLK, 2, WBIG_COLS], BF16, tag="wbig_sb")
            nc.sync.dma_start(out=wbig_sb[:], in_=wbig_d[:])
            whh8_sb = pool.tile([BLK, 2, G3], FP8, tag="whh8_sb")
            nc.sync.dma_start(out=whh8_sb[:], in_=whh8_d[:])
            tf_sb = pool.tile([16, 20], F32, tag="tf_sb")
            nc.sync.dma_start(out=tf_sb[:], in_=tf_d[:])
            # Pool queue: stage-B weights (parallel with at/xh on SP).
            wes_sb = pool.tile([BLK, NB, EMB], BF16, tag="wes_sb")
            nc.gpsimd.dma_start(out=wes_sb[:], in_=wes_d[:])
            return at_sb, xh_sb, wes_sb, wbig_sb, whh8_sb, cb_sb, br_sb, tf_sb

        if not loads_in_body:
            (at_sb, xh_sb, wes_sb, wbig_sb, whh8_sb, cb_sb, br_sb,
             tf_sb) = emit_loads(const)

        # warm the ACT sigmoid/tanh table set while DMAs run
        dummy = const.tile([BLK, 1], F32)
        nc.vector.memset(dummy[:], 0.0)
        dummy2 = const.tile([BLK, 1], F32)
        nc.scalar.activation(dummy2[:], dummy[:], AF.Sigmoid)

        for _ in range(repeat):
            if loads_in_body:
                (at_sb, xh_sb, wes_sb, wbig_sb, whh8_sb, cb_sb, br_sb,
                 tf_sb) = emit_loads(work)
            ident = cb_sb[:, 0:8, :]          # [128, 128] identity (bf16)
            bhn_bc = cb_sb[:, 8, 0:8]         # [128, (2c x 4b)] bhh_n bcast
            h0_c = [cb_sb[:, 8, 8:12], cb_sb[:, 8, 12:16]]   # per-kc [128, 4]
            wf0_c = [cb_sb[:, 9, :], cb_sb[:, 10, :]]        # per-kc [128, 16]
            ones48 = br_sb[0:1, 1280:1328]                   # [1, 48] of 1.0
            def brow(k):                                     # [1, 128] bias row
                return br_sb[0:1, k * BLK:(k + 1) * BLK]

            # ---- stage A: h1T blocks [128, 48] = A @ xfT (block tridiagonal)
            h1_sb = work.tile([BLK, NB, BT], BF16)
            for i in range(NB):
                ps = psA.tile([BLK, BT], F32, tag="psa")
                for jo in range(3):
                    nc.tensor.matmul(
                        ps[:], at_sb[:, 3 * i + jo, :], xh_sb[:, i + jo, :],
                        start=(jo == 0), stop=(jo == 2))
                if i % 2 == 0:
                    nc.vector.tensor_copy(h1_sb[:, i, :], ps[:])
                else:
                    nc.scalar.activation(h1_sb[:, i, :], ps[:], AF.Copy)

            # ---- stage B: partial t2preT [256, 48] = W_es @ h1 (one PSUM tile)
            psb = psB.tile([BLK, 2, BT], F32, tag="psb")
            for e in range(2):
                for i in range(NB):
                    nc.tensor.matmul(
                        psb[:, e, :], wes_sb[:, i, e * BLK:(e + 1) * BLK],
                        h1_sb[:, i, :], start=(i == 0), stop=False)
                nc.tensor.matmul(psb[:, e, :], brow(e), ones48,
                                 start=False, stop=True)
            t2p_sb = work.tile([BLK, 2, BT], BF16)
            nc.vector.tensor_copy(t2p_sb[:], psb[:])

            # ---- combine partials across the 8 cores
            if coll == "ar":
                cc_in = dram.tile([BLK, 2, BT], F32, tag="cc_in")
                cc_out = dram.tile([BLK, 2, BT], F32, tag="cc_out")
                t2pf_sb = work.tile([BLK, 2, BT], F32, tag="t2pf")
                nc.vector.tensor_copy(t2pf_sb[:], psb[:])
                nc.gpsimd.dma_start(out=cc_in[:], in_=t2pf_sb[:])
                nc.gpsimd.collective_compute(
                    "AllReduce", ALU.add,
                    replica_groups=[list(range(N_CORES))],
                    ins=[cc_in.opt()], outs=[cc_out.opt()])
                t2r_sb = work.tile([BLK, 2, BT], F32, tag="t2r")
                nc.gpsimd.dma_start(out=t2r_sb[:], in_=cc_out[:])
                t2pre = t2r_sb
            else:
                cc_in = dram.tile([BLK, 2, BT], BF16, tag="cc_in")
                cc_out = dram.tile([N_CORES, BLK, 2, BT], BF16, tag="cc_out")
                nc.gpsimd.dma_start(out=cc_in[:], in_=t2p_sb[:])
                if coll == "ag":
                    nc.gpsimd.collective_compute(
                        "AllGather", ALU.bypass,
                        replica_groups=[list(range(N_CORES))],
                        ins=[cc_in.opt()], outs=[cc_out.opt()])
                else:   # timing stand-in: local copy into slot 0
                    nc.gpsimd.dma_start(out=cc_out[0], in_=cc_in[:])
                gat_sb = work.tile([BLK, N_CORES, 2 * BT], BF16, tag="gat")
                if coll == "ag":
                    nc.gpsimd.dma_start(
                        out=gat_sb[:], in_=cc_out[:].transpose([1, 0, 2, 3]))
                else:
                    for r in range(N_CORES):
                        nc.gpsimd.dma_start(out=gat_sb[:, r, :], in_=cc_out[0])
                # tree reduce the 8 partials (f32 accumulation)
                r1 = work.tile([BLK, 4, 2 * BT], BF16, tag="red1")
                nc.vector.tensor_add(r1[:], gat_sb[:, 0:4, :], gat_sb[:, 4:8, :])
                r2 = work.tile([BLK, 2, 2 * BT], BF16, tag="red2")
                nc.vector.tensor_add(r2[:], r1[:, 0:2, :], r1[:, 2:4, :])
                t2r_sb = work.tile([BLK, 2, BT], F32, tag="t2r")
                nc.vector.tensor_add(t2r_sb[:], r2[:, 0, :], r2[:, 1, :])
                t2pre = t2r_sb

            # sigmoid(t2pre); b_emb/8 was seeded into each partial pre-gather
            t2_sb = work.tile([BLK, 2, BT], BF16)
            nc.scalar.activation(t2_sb[:], t2pre[:], AF.Sigmoid)

            # ---- stage C: t4T = sigmoid(W_l1 @ t2T + b_l1)
            psc = psB.tile([BLK, 2, BT], F32, tag="psc")
            for mc in range(2):
                nc.tensor.matmul(psc[:, mc, :], brow(2 + mc), ones48,
                                 start=True, stop=False)
                for kc in range(2):
                    nc.tensor.matmul(
                        psc[:, mc, :], wbig_sb[:, kc, WL1_OFF + mc * BLK:
                                               WL1_OFF + (mc + 1) * BLK],
                        t2_sb[:, kc, :], start=False, stop=(kc == 1))
            t4_sb = work.tile([BLK, 2, BT], BF16)
            nc.scalar.activation(t4_sb[:], psc[:], AF.Sigmoid)

            # ---- stage D: xpT [128, 6, 4, 12] = W_ih @ t4T (+ gate biases)
            ps_xp = psX.tile([BLK, 6, B, T], F32)
            for c in range(6):
                nc.tensor.matmul(ps_xp[:, c, :, :], brow(4 + c), ones48,
                                 start=True, stop=False)
                for kc in range(2):
                    nc.tensor.matmul(
                        ps_xp[:, c, :, :],
                        wbig_sb[:, kc, WIH_OFF + c * BLK:WIH_OFF + (c + 1) * BLK],
                        t4_sb[:, kc, :], start=False, stop=(kc == 1))
            # rz chunks (bias included) in bf16 for the PSUM seeding mms;
            # n chunks (bias bih only) kept f32 for the nin add
            xp_sb = work.tile([BLK, 4, B, T], BF16)
            nc.vector.tensor_scalar(xp_sb[:], ps_xp[:, 0:4, :, :],
                                    WHH_SCALE, 0.0,
                                    op0=ALU.mult, op1=ALU.add)
            xn_sb = work.tile([BLK, 2, B, T], F32)
            nc.scalar.activation(xn_sb[:], ps_xp[:, 4:6, :, :], AF.Copy,
                                 scale=WHH_SCALE)

            # ---- GRU over T steps
            h_prev_c = h0_c                       # per-kc [128, 4] APs
            h_prev_full = cb_sb[:, 8, 8:16]       # [128, 8] AP (same data)
            for t in range(t_steps):
                ps_g = psG.tile([BLK, 6, B], F32, tag="ps_g")
                # PSUM seeds (no h dependency): xp_rz into chunks 0-3,
                # bhh_n broadcast into chunks 4-5 -- identity-weight matmuls
                nc.tensor.matmul(ps_g[:, 0:4, :], ident, xp_sb[:, :, :, t],
                                 start=True, stop=False,
                                 skip_group_check=True)
                nc.tensor.matmul(ps_g[:, 4:6, :], ident, cb_sb[:, 8, 0:8],
                                 start=True, stop=False,
                                 skip_group_check=True)
                # W_hh @ h (fp8 weights, x8): rz-gate chunks first so the
                # merged sigmoid can start while the n-gate matmuls finish
                for c in (0, 1, 2, 3, 4, 5):
                    for kc in range(2):
                        nc.tensor.matmul(
                            ps_g[:, c, :],
                            whh8_sb[:, kc, c * BLK:(c + 1) * BLK],
                            h_prev_c[kc], start=False, stop=(kc == 1),
                            skip_group_check=True)
                # Emission order matters: a consumer's semaphore wait covers
                # every producer-engine op emitted before it, so each op is
                # emitted as late as its true dependency allows.
                rz_sb = gru.tile([BLK, 4, B], F32, tag="rz")
                nc.scalar.activation(rz_sb[:], ps_g[:, 0:4, :], AF.Sigmoid,
                                     scale=1.0 / WHH_SCALE)
                # n pre-activation: nin = xn + r * (hn + bhh_n)
                npre = gru.tile([BLK, 2, B], F32, tag="npre")
                nc.vector.tensor_mul(npre[:], ps_g[:, 4:6, :], rz_sb[:, 0:2, :])
                nin = gru.tile([BLK, 2, B], F32, tag="nin")
                nc.vector.tensor_add(nin[:], npre[:], xn_sb[:, :, :, t])
                nw = gru.tile([BLK, 2, B], F32, tag="nw")
                nc.scalar.activation(nw[:], nin[:], AF.Tanh,
                                     scale=1.0 / WHH_SCALE)
                # u = z*h and v = 1-z run on the (otherwise idle) GpSimd
                # engine during the tanh, so the DVE counter that gates the
                # tanh stops at nin
                u = gru.tile([BLK, 2, B], F32, tag="u")
                nc.gpsimd.tensor_mul(u[:], rz_sb[:, 2:4, :], h_prev_full)
                v = gru.tile([BLK, 2, B], F32, tag="v")
                nc.gpsimd.tensor_scalar(v[:], rz_sb[:, 2:4, :], -1.0, 1.0,
                                        op0=ALU.mult, op1=ALU.add)
                w = gru.tile([BLK, 2, B], F32, tag="w")
                nc.vector.tensor_mul(w[:], nw[:], v[:])
                h_new = gru.tile([BLK, 2, B], BF16, tag="h")
                nc.vector.tensor_add(h_new[:], w[:], u[:])
                h_prev_c = [h_new[:, 0, :], h_new[:, 1, :]]
                h_prev_full = h_new[:]

            # ---- tail MLP: [4,256] -> 16 -> 16 -> 1, sigmoid each
            ps_o1 = psO.tile([16, B], F32, tag="o")
            for kc in range(2):
                nc.tensor.matmul(ps_o1[:], wf0_c[kc], h_prev_c[kc],
                                 start=(kc == 0), stop=(kc == 1))
            o1 = work.tile([16, B], F32, tag="o1s")
            nc.scalar.activation(o1[:], ps_o1[:], AF.Sigmoid,
                                 bias=tf_sb[:, 17:18])
            ps_o2 = psO.tile([16, B], F32, tag="o")
            nc.tensor.matmul(ps_o2[:], tf_sb[:, 0:16], o1[:],
                             start=True, stop=True)
            o2 = work.tile([16, B], F32, tag="o2s")
            nc.scalar.activation(o2[:], ps_o2[:], AF.Sigmoid,
                                 bias=tf_sb[:, 18:19])
            ps_o3 = psO.tile([1, B], F32, tag="o")
            nc.tensor.matmul(ps_o3[:], tf_sb[:, 16:17], o2[:],
                             start=True, stop=True)
            o3 = work.tile([1, B], F32, tag="o3s")
            nc.scalar.activation(o3[:], ps_o3[:], AF.Sigmoid,
                                 bias=tf_sb[0:1, 19:20])
            nc.sync.dma_start(out=out_d[:], in_=o3[:])

    nc.compile()
    return nc


# ---------------------------------------------------------------- host side

def prepare_in_maps(x, h0, rows, cols, W_emb, b_emb, W_l1, b_l1,
                    W_ih, W_hh, b_ih, b_hh, W_f0, b_f0, W_f1, b_f1,
                    W_f2, b_f2):
    import ml_dtypes
    f32 = np.float32
    bf = ml_dtypes.bfloat16
    x = np.ascontiguousarray(x, f32)
    assert int(rows.max()) < SUP and int(cols.max()) < SUP

    # dense banded adjacency on its true support (duplicates sum = coalesce)
    A = np.zeros((SUP, SUP), f32)
    np.add.at(A, (np.asarray(rows), np.asarray(cols)), 1.0)

    S_pad = NBLOCKS * BLK
    ATp = np.zeros((S_pad, S_pad), f32)
    ATp[:SUP, :SUP] = A.T

    xf = x.reshape(BT, N)
    # xsT padded with one leading zero block (halo for core 0) + tail blocks
    XTp = np.zeros(((NBLOCKS + 2) * BLK, BT), f32)
    XTp[BLK:BLK + SUP] = xf[:, :SUP].T

    WesT = np.zeros((S_pad, EMB), f32)
    WesT[:SUP] = np.asarray(W_emb, f32)[:, :SUP].T

    def pm(vec, k):  # partition-major [128, k] view of a length 128*k vector
        return np.ascontiguousarray(np.asarray(vec, f32).reshape(k, BLK).T)

    def pm3(w, k):  # [M, K] weight -> partition-major [128, K//128, M]
        return np.ascontiguousarray(
            np.asarray(w, f32).T.reshape(-1, BLK, k).transpose(1, 0, 2))

    bih = np.asarray(b_ih, f32)
    bhh = np.asarray(b_hh, f32)
    bxp = np.concatenate([bih[:512] + bhh[:512], bih[512:]])  # rz: both, n: ih

    # wbig pack [128, 2, 1024]: wl1 | wih
    wbig = np.zeros((BLK, 2, WBIG_COLS), f32)
    wbig[:, :, WL1_OFF:WL1_OFF + EMB] = pm3(W_l1, EMB)
    wbig[:, :, WIH_OFF:WIH_OFF + G3] = pm3(W_ih, G3)
    # whh in fp8-e4m3 scaled x8 (moves its values out of the denormal
    # range); the GRU descales by 1/8 inside the ACT sigmoid/tanh
    import ml_dtypes
    whh8 = (pm3(W_hh, G3) * 8.0).astype(ml_dtypes.float8_e4m3)

    # cb pack [128, 11, 16] bf16: identity | bhn_bc + h0 | wf0 (2 chunks)
    cb = np.zeros((BLK, 11, 16), f32)
    cb[:, 0:8, :] = np.eye(BLK, dtype=f32).reshape(BLK, 8, 16)
    bhn = bhh[512:].reshape(2, BLK) * 8.0              # [c, p], x8 (descaled
    cb[:, 8, 0:8] = np.repeat(bhn.T, B, axis=1).reshape(BLK, 8)  # in ACT)
    h0c = np.asarray(h0, f32)[0].T.reshape(2, BLK, B)  # [kc, p, b]
    cb[:, 8, 8:16] = h0c.transpose(1, 0, 2).reshape(BLK, 8)
    wf0t = pm3(W_f0, 16)                               # [128, 2, 16]
    cb[:, 9, :] = wf0t[:, 0, :]
    cb[:, 10, :] = wf0t[:, 1, :]

    # br pack [1, 1328] bf16: bias rows for the ones-matmul PSUM seeds.
    # bemb is divided by 8 because every core seeds it into its partial and
    # the cross-core reduce sums 8 copies.
    br = np.zeros((1, 1328), f32)
    br[0, 0:256] = np.asarray(b_emb, f32) / 8.0
    br[0, 256:512] = np.asarray(b_l1, f32)
    br[0, 512:1280] = bxp
    br[0, 1280:1328] = 1.0

    # tf pack [16, 20] f32: wf1 | wf2t | bf0 | bf1 | bf2
    tf = np.zeros((16, 20), f32)
    tf[:, 0:16] = np.asarray(W_f1, f32).T
    tf[:, 16] = np.asarray(W_f2, f32).reshape(16)
    tf[:, 17] = np.asarray(b_f0, f32)
    tf[:, 18] = np.asarray(b_f1, f32)
    tf[0, 19] = np.asarray(b_f2, f32).reshape(1)[0]

    common = dict(wbig=wbig.astype(bf), whh8=whh8, cb=cb.astype(bf),
                  br=br.astype(bf), tf=tf)

    in_maps = []
    for c in range(N_CORES):
        at = np.zeros((3 * NB, BLK, BLK), f32)
        for i in range(NB):
            I = NB * c + i
            for jo in range(3):
                J = I - 1 + jo
                if 0 <= J < NBLOCKS:
                    at[3 * i + jo] = ATp[J * BLK:(J + 1) * BLK,
                                         I * BLK:(I + 1) * BLK]
        xh = np.ascontiguousarray(
            XTp[NB * c * BLK:(NB * c + NB + 2) * BLK]
            .reshape(NB + 2, BLK, BT).transpose(1, 0, 2))
        wes = np.ascontiguousarray(
            WesT[NB * c * BLK:(NB * (c + 1)) * BLK]
            .reshape(NB, BLK, EMB).transpose(1, 0, 2))
        in_maps.append(dict(
            at=np.ascontiguousarray(at.transpose(1, 0, 2)).astype(bf),
            xh=xh.astype(bf), wes=wes.astype(bf), **common))
    return in_maps


# production configuration for kernel(); test.py reads this too
KERNEL_CONFIG = dict(coll="ag")

_CACHE = {}


def kernel(**inputs) -> np.ndarray:
    if "nc" not in _CACHE:
        _CACHE["nc"] = build_program(**KERNEL_CONFIG)
    nc = _CACHE["nc"]
    in_maps = prepare_in_maps(**inputs)
    res = run_bass_kernel_spmd(nc, in_maps, list(range(N_CORES)))
    out = res.results[0]["out"]          # [1, 4]
    return np.ascontiguousarray(out.T.astype(np.float32))  # [4, 1]


if __name__ == "__main__":
    import importlib.util
    spec = importlib.util.spec_from_file_location("reference", "reference.py")
    ref = importlib.util.module_from_spec(spec)
    spec.loader.exec_module(ref)
    inputs = {k: np.asarray(v) for k, v in ref.setup_inputs().items()}
    expected = np.asarray(ref.reference(**inputs))
    got = kernel(**inputs)
    err = np.abs(got - expected).max() / np.abs(expected).max()
    print("expected:", expected.ravel())
    print("got:     ", got.ravel())
    print("Relative error:", err)
